# revision 47
# baseline (speedup 1.0000x reference)
"""GroupedQueryAttention (B=2,T=2048,D=2048,HQ=16,HKV=8,HD=128) on 8 trn2 cores.

Sharding: 2-way data-parallel over batch x 4-way tensor-parallel over KV
groups.  Core c: batch c//4, KV-group c%4 (4 Q heads, 2 KV heads).  Each
core computes a partial [T, D] output (its heads' contribution through
out_proj) in bf16; the host sums the 4 TP partials per batch in f32.

Design (all matmul operands bf16, PSUM f32):
- Q/K projected directly in transposed [d, t] layout (weight chunks as
  lhsT, resident x^T supertile as rhs) -> no PE transposes and no
  per-head PSUM->SBUF copies; V projected in [t, e] (x^T chunks as lhsT).
- RMS-norm stats via ACT Square + ones-column matmul (partition reduce);
  1/rms = ACT Sqrt + DVE reciprocal_approx_fast; the inverse row is
  partition-broadcast by gpsimd and folded into the rope tail multiply.
  The stats matmul of head i issues while head i+1's projection chain
  streams, so the PE never waits on the ACT square.
- rope in [d, t] layout with q/k_scale and the rotate-half sign baked
  into transposed cos/sin tables (host-precomputed, bf16).
- attention in ST layout: S^T chunks [k,q] -> ACT exp -> P^T (bf16)
  feeds ctx^T; the softmax denominator is a bf16 DVE running sum of the
  P^T chunks + ONE ones-column matmul per (head, supertile) (instead of
  re-streaming every chunk through the PE), deferred into the next
  head's S phase so it never waits on the DVE add chain; division via
  DVE reciprocal_approx_fast + gpsimd partition_broadcast, applied once
  to ctx^T.  Causality at 128-chunk granularity + 0/1 triangle multiply
  on the diagonal chunk.  S runs 2 chunks ahead of exp (stp bufs=3).
- HAM warmup matmuls at t=0 + startup DMAs spread across 4 engine
  queues so the first projections start at 2.4 GHz as soon as their
  chunks land.
- software-pipelined emission: supertile st's attention interleaves the
  previous supertile's out-projection (per head) and the next
  supertile's projection jobs, keeping the PE dense while ACT paces the
  exp chain.  Startup DMAs are chunked and dual-queued.
"""

import numpy as np
import ml_dtypes

import concourse.bass as bass
import concourse.bacc as bacc
import concourse.mybir as mybir
from concourse.tile import TileContext

F32 = mybir.dt.float32
F32R = mybir.dt.float32r
BF16 = mybir.dt.bfloat16
AFT = mybir.ActivationFunctionType
ALU = mybir.AluOpType

B, T, D = 2, 2048, 2048
HQ, HKV, HD = 16, 8, 128
EPS = 1e-6
NCORES = 8
HQL, HKVL = 4, 2
EQ, EKV = HQL * HD, HKVL * HD   # 512, 256
ND = D // 128
NST = 4
TPS = 4
NTT = NST * TPS
ISQ = float(1.0 / np.sqrt(np.float32(HD)))

_compiled = None


def _build():
    nc = bacc.Bacc()
    xT = nc.declare_dram_parameter("xT", [D, T], BF16, isOutput=False)
    wq = nc.declare_dram_parameter("wq", [D, EQ], BF16, isOutput=False)
    wkv = nc.declare_dram_parameter("wkv", [D, EQ], BF16, isOutput=False)
    wo = nc.declare_dram_parameter("wo", [EQ, D], BF16, isOutput=False)
    ctq_d = nc.declare_dram_parameter("ctq", [HD, T], BF16, isOutput=False)
    stq_d = nc.declare_dram_parameter("stq", [HD, T], BF16, isOutput=False)
    ones_d = nc.declare_dram_parameter("ones_col", [128, 1], BF16, isOutput=False)
    ut01_d = nc.declare_dram_parameter("ut01", [128, 128], BF16, isOutput=False)
    out = nc.declare_dram_parameter("out", [T, D], BF16, isOutput=True)

    with TileContext(nc) as tc:
        with (
            nc.allow_low_precision(reason="bf16 matmuls, bf16 softmax tiles"),
            tc.tile_pool(name="res", bufs=1) as res,
            tc.tile_pool(name="work", bufs=2) as work,
            tc.tile_pool(name="pp", bufs=1, space="PSUM") as pp,
        ):
            wq_sb = res.tile([128, ND * EQ], BF16, name="wq_sb")
            wkv_sb = res.tile([128, ND * EQ], BF16, name="wkv_sb")
            wo_sb = res.tile([128, HQL * D], BF16, name="wo_sb")
            ktr_sb = res.tile([128, HKVL * T], BF16, name="ktr_sb")
            vv_sb = res.tile([128, NTT * EKV], BF16, name="vv_sb")
            ctq = res.tile([128, T], BF16, name="ctq")
            stq = res.tile([128, T], BF16, name="stq")
            ones_col = res.tile([128, 1], BF16, name="ones_col")
            ut01 = res.tile([128, 128], BF16, name="ut01")
            zero128 = res.tile([128, 1], F32, name="zero128")
            ones_f32r = res.tile([128, 1], F32, name="ones_f32r")
            nc.vector.memset(ones_f32r[:], 1.0)
            eps1 = res.tile([1, 1], F32, name="eps1")
            nc.vector.memset(zero128[:], 0.0)
            nc.vector.memset(eps1[:], EPS)

            # HAM warmup: keep the PE busy while the startup DMAs land so
            # the real matmuls start at 2.4 GHz instead of 1.2.  Streams
            # uninitialized wo_sb garbage into a never-read PSUM tile;
            # wo's DMA is last in the startup order, so the WAR edge on
            # these reads costs nothing.
            wrm = pp.tile([128, 512], F32, tag="proj", bufs=2, name="warmpp")
            NWARM = 7
            for i in range(NWARM):
                nc.tensor.matmul(wrm[:], wo_sb[:, 0:128], wo_sb[:, 0:512],
                                 start=(i == 0), stop=(i == NWARM - 1))

            # startup DMAs: interleave x^T(st=0) chunks with Wq chunks so
            # the first projection matmuls can begin ASAP; everything else
            # lands while the first projections run.
            wqr = wq.rearrange("(j p) e -> p j e", p=128)
            wkvr = wkv.rearrange("(j p) e -> p j e", p=128)
            x0 = work.tile([128, ND * 512], BF16, tag="xst", bufs=2,
                           name="xst_0")
            x0r = x0[:].rearrange("p (j t) -> p j t", j=ND)
            xr0 = xT.rearrange("(j p) t -> p j t", p=128)[:, :, 0:512]
            wq_v = wq_sb[:].rearrange("p (j e) -> p j e", j=ND)
            wkv_v = wkv_sb[:].rearrange("p (j e) -> p j e", j=ND)
            # startup DMAs across the 3 DMA-capable queues: x on sync
            # (kept clear for the xst streams), wq on scalar, everything
            # else on gpsimd in consumption order.  ut01 and wo go last —
            # they are needed late and their descriptor-gen is expensive.
            for j0, jw in ((0, 2), (2, 2), (4, 4), (8, 4), (12, 4)):
                nc.sync.dma_start(out=x0r[:, j0:j0 + jw],
                                  in_=xr0[:, j0:j0 + jw])
                nc.scalar.dma_start(out=wq_v[:, j0:j0 + jw],
                                    in_=wqr[:, j0:j0 + jw])
            nc.gpsimd.dma_start(out=ones_col[:], in_=ones_d[:])
            # Gate the rest of the gpsimd DMA stream on x0 being nearly
            # landed: HBM bandwidth (~358 GB/s/core) is shared across
            # queues, and x0/wq are the tensors the first projections
            # block on.  The gate is a WAW data dep: a tiny broadcast of
            # x0's last chunk into each DMA's destination, so the
            # scheduler cannot hoist the DMA ahead of it.
            x0tail = x0[0:1, 15 * 512:15 * 512 + 16]

            def dgate(dst16):
                nc.gpsimd.partition_broadcast(dst16, x0tail)

            # rope tables first: the q-rope DVE muls free the proj PSUM
            # buffers that the k projections block on
            dgate(ctq[:, 0:16])
            nc.gpsimd.dma_start(out=ctq[:], in_=ctq_d[:])
            dgate(stq[:, 0:16])
            nc.gpsimd.dma_start(out=stq[:], in_=stq_d[:])
            for j0 in range(0, ND, 4):
                dgate(wkv_sb[:, j0 * 512:j0 * 512 + 16])
                nc.gpsimd.dma_start(out=wkv_v[:, j0:j0 + 4],
                                    in_=wkvr[:, j0:j0 + 4])
            nc.gpsimd.dma_start(out=ut01[:], in_=ut01_d[:])

            xst = [None] * NST
            xst[0] = x0
            qtr = [None] * HQL
            csb = {}
            csb2 = {}

            def load_xst(st, gate16=None):
                t = work.tile([128, ND * 512], BF16, tag="xst", bufs=2,
                              name=f"xst_{st}")
                tr = t[:].rearrange("p (j t) -> p j t", j=ND)
                xr = xT.rearrange("(j p) t -> p j t",
                                  p=128)[:, :, st * 512:(st + 1) * 512]
                for j0 in range(0, ND, 4):
                    if gate16 is not None:
                        nc.gpsimd.partition_broadcast(
                            t[:, j0 * 512:j0 * 512 + 16], gate16)
                    nc.sync.dma_start(out=tr[:, j0:j0 + 4], in_=xr[:, j0:j0 + 4])
                xst[st] = t

            def proj_head(st, wsb, eoff, name):
                """16-chunk projection matmul into a [128,512] PSUM tile."""
                prj = pp.tile([128, 512], F32, tag="proj", bufs=2, name=name)
                for j in range(ND):
                    nc.tensor.matmul(
                        prj[:],
                        wsb[:, j * EQ + eoff:j * EQ + eoff + 128],
                        xst[st][:, j * 512:(j + 1) * 512],
                        start=(j == 0), stop=(j == ND - 1))
                return prj

            def rope_early(st, hh, prj, ct, st_t, acc):
                """ACT square + inv-independent rope part."""
                u = f"{st}_{hh}"
                c0 = st * 512
                sq = work.tile([128, 512], BF16, tag="sq", bufs=3, name=f"sq_{u}")
                nc.scalar.activation(sq[:], prj[:], AFT.Square,
                                     bias=zero128[:, 0:1])
                tmp = work.tile([128, 512], BF16, tag="tmp", bufs=2,
                                name=f"tmp_{u}")
                nc.vector.tensor_mul(tmp[0:64, :], prj[64:128, :],
                                     st_t[0:64, c0:c0 + 512])
                nc.vector.tensor_mul(tmp[64:128, :], prj[0:64, :],
                                     st_t[64:128, c0:c0 + 512])
                nc.vector.tensor_mul(acc[:], prj[:], ct[:, c0:c0 + 512])
                nc.vector.tensor_add(acc[:], acc[:], tmp[:])
                return sq

            def stats_late(st, hh, sq, cat_row, cslot):
                u = f"{st}_{hh}"
                stat = pp.tile([1, 512], F32, tag="row", bufs=1, name=f"st_{u}")
                nc.tensor.matmul(stat[:], ones_col[:], sq[:], start=True,
                                 stop=True)
                nc.scalar.activation(cat_row[0:1, cslot:cslot + 512], stat[:],
                                     AFT.Sqrt, bias=eps1[:, 0:1], scale=1.0 / HD)

            def inv_chain(cat, c0, width, rows, invb, io):
                """invb[:, io:io+width] = broadcast(1/cat[0, c0:c0+width])."""
                nc.vector.reciprocal_approx_fast(rows[0:1, c0:c0 + width],
                                                 cat[0:1, c0:c0 + width])
                nc.gpsimd.partition_broadcast(invb[:, io:io + width],
                                              rows[0:1, c0:c0 + width])

            def phase1_jobs(st):
                """Return a list of emission closures (jobs) for supertile
                st's projections+norm+rope; run in order, possibly
                interleaved into phase2's head loop."""
                cat = work.tile([1, 3584], F32, tag="cat", bufs=1,
                                name=f"cat_{st}")
                rows = work.tile([1, 3584], F32, tag="rows", bufs=1,
                                 name=f"rows_{st}")
                invb = work.tile([128, 3072], F32, tag="invb", bufs=1,
                                 name=f"invb_{st}")
                pend = []

                def flush_pend():
                    if not pend:
                        return
                    kind, idx, sq2, acc2 = pend.pop(0)
                    cslot = idx * 512 if kind == "q" else 2048 + idx * 512
                    stats_late(st, f"{kind}{idx}", sq2, cat, cslot)
                    inv_chain(cat, cslot, 512, rows, invb, cslot)
                    if kind == "q":
                        q = work.tile([128, 512], BF16, tag=f"qtr{idx}", bufs=2,
                                      name=f"qtr_{st}_{idx}")
                        nc.vector.tensor_mul(q[:], acc2[:],
                                             invb[:, cslot:cslot + 512])
                        qtr[idx] = q
                    else:
                        nc.vector.tensor_mul(
                            ktr_sb[:, idx * T + st * 512:idx * T + (st + 1) * 512],
                            acc2[:], invb[:, cslot:cslot + 512])

                def qk_job(kind, idx):
                    def run():
                        if kind == "q":
                            prj = proj_head(st, wq_sb, idx * 128,
                                            f"qp_{st}_{idx}")
                            acc = work.tile([128, 512], BF16, tag=f"qacc{idx}",
                                            bufs=2, name=f"qacc_{st}_{idx}")
                            sq = rope_early(st, f"q{idx}", prj, ctq, stq, acc)
                        else:
                            prj = proj_head(st, wkv_sb, idx * 128,
                                            f"kp_{st}_{idx}")
                            acc = work.tile([128, 512], BF16, tag=f"kacc{idx}",
                                            bufs=2, name=f"kacc_{st}_{idx}")
                            sq = rope_early(st, f"k{idx}", prj, ctq, stq, acc)
                        # depth-2 pending queue: the stats matmul flushes
                        # two proj chains after its Square was emitted, so
                        # it never waits on the ACT backlog
                        if len(pend) >= 2:
                            flush_pend()
                        pend.append((kind, idx, sq, acc))
                    return run

                def v_job(tq):
                    def run():
                        c = st * TPS + tq
                        vp = pp.tile([128, 512], F32, tag="proj", bufs=2,
                                     name=f"vp_{st}_{tq}")
                        for j in range(ND):
                            nc.tensor.matmul(
                                vp[:, 0:EKV],
                                xst[st][:, j * 512 + tq * 128:
                                        j * 512 + (tq + 1) * 128],
                                wkv_sb[:, j * EQ + EKV:(j + 1) * EQ],
                                start=(j == 0), stop=(j == ND - 1))
                        nc.scalar.copy(vv_sb[:, c * EKV:(c + 1) * EKV],
                                       vp[:, 0:EKV])
                        if tq == 0:
                            flush_pend()
                        elif tq == 1:
                            while pend:
                                flush_pend()
                    return run

                def qq012_job():
                    """q0+q1+q2 chunk-major: the startup is DMA-paced, so
                    interleave three heads' j-chunks to consume each
                    arriving x/wq chunk with 3x the PE work."""
                    prjs = [pp.tile([128, 512], F32, tag="proj", bufs=2,
                                    name=f"qqp_{i}") for i in range(2)]
                    prjs.append(pp.tile([128, 512], F32, tag="stp", bufs=3,
                                        name="qqp_2"))
                    for j in range(ND):
                        for i in range(3):
                            nc.tensor.matmul(
                                prjs[i][:],
                                wq_sb[:, j * EQ + i * 128:j * EQ + i * 128 + 128],
                                xst[0][:, j * 512:(j + 1) * 512],
                                start=(j == 0), stop=(j == ND - 1))
                    for i in range(3):
                        acc = work.tile([128, 512], BF16, tag=f"qacc{i}",
                                        bufs=2, name=f"qacc_0_{i}")
                        sq = rope_early(0, f"q{i}", prjs[i], ctq, stq, acc)
                        if len(pend) >= 2:
                            flush_pend()
                        pend.append(("q", i, sq, acc))

                phase1.rows = rows
                if st == 0:
                    jobs = [qq012_job]
                    jobs += [qk_job("q", h) for h in range(3, HQL)]
                else:
                    jobs = [qk_job("q", h) for h in range(HQL)]
                jobs += [qk_job("k", g) for g in range(HKVL)]
                jobs += [v_job(tq) for tq in range(TPS)]
                return jobs

            def phase1(st):
                for j in phase1_jobs(st):
                    j()

            def phase2(st, zip3=None, jobs1=None):
                nch = TPS * (st + 1)
                jobs1 = list(jobs1 or [])
                per = (len(jobs1) + HQL - 1) // HQL if jobs1 else 0
                qcur = list(qtr)
                pend_tail = []
                # previous supertile's out-proj, interleaved one [128,512]
                # block at a time through the chunk loop: those ~900ns PE
                # fillers cover the ACT exp deficit (exp ~571ns/chunk vs
                # ~426ns of S+ctx PE work).
                zjobs = phase3_jobs(zip3) if zip3 is not None else []
                znum = len(zjobs)
                zticks = HQL * nch
                zctr = [0]

                def ztick():
                    zctr[0] += 1
                    # proportional pacing: job k fires at tick k*zticks/znum
                    while zjobs and zctr[0] * znum >= \
                            (znum - len(zjobs) + 1) * zticks:
                        zjobs.pop(0)()

                def flush_tail():
                    if pend_tail:
                        pend_tail.pop()()

                for h in range(HQL):
                    g = h // 2
                    ctxp = pp.tile([128, 512], F32, tag="ctx", bufs=2,
                                   name=f"ctx_{st}_{h}")
                    # bf16 running sum of P^T chunks on the DVE; the final
                    # [1,512] denominator is ONE ones-column matmul instead
                    # of re-streaming every chunk through the PE.  bf16 is
                    # enough: the f32 partition-reduce averages the
                    # independent per-entry roundings (all-positive sums).
                    pacc = work.tile([128, 512], BF16, tag="pacc", bufs=2,
                                     name=f"pacc_{st}_{h}")
                    pts = [None] * nch

                    def s_exp(ci):
                        dj = ci - TPS * st
                        qlo = 128 * dj if dj > 0 else 0
                        w = 512 - qlo
                        stp = pp.tile([128, 512], F32, tag="stp", bufs=3,
                                      name=f"stp_{st}_{h}_{ci}")
                        nc.tensor.matmul(
                            stp[:, :w],
                            ktr_sb[:, g * T + ci * 128:g * T + (ci + 1) * 128],
                            qcur[h][:, qlo:512],
                            start=True, stop=True)
                        pt = work.tile([128, 512], BF16, tag="pt", bufs=4,
                                       name=f"pt_{st}_{h}_{ci}")
                        nc.scalar.activation(pt[:, :w], stp[:, :w], AFT.Exp,
                                             bias=zero128[:, 0:1], scale=ISQ)
                        if dj >= 0:
                            nc.vector.tensor_mul(pt[:, :128], pt[:, :128],
                                                 ut01[:])
                        if ci == 0:
                            nc.vector.tensor_copy(pacc[:], pt[:])
                        else:
                            nc.vector.tensor_add(pacc[:, qlo:512],
                                                 pacc[:, qlo:512], pt[:, :w])
                        pts[ci] = pt

                    def ctx_rsp(ci):
                        dj = ci - TPS * st
                        qlo = 128 * dj if dj > 0 else 0
                        w = 512 - qlo
                        first, last = ci == 0, ci == nch - 1
                        nc.tensor.matmul(
                            ctxp[:, qlo:512],
                            vv_sb[:, ci * EKV + g * 128:ci * EKV + (g + 1) * 128],
                            pts[ci][:, :w],
                            start=first, stop=last)
                        pts[ci] = None

                    la = min(2, nch - 1)
                    for ci in range(la):
                        s_exp(ci)
                    # previous head's denominator + normalize, deferred here
                    # so its rsp matmul never waits on the DVE add chain
                    flush_tail()
                    for ci in range(la, nch):
                        s_exp(ci)
                        ctx_rsp(ci - la)
                        ztick()
                    for ci in range(nch - la, nch):
                        ctx_rsp(ci)
                        ztick()

                    def tail(h=h, ctxp=ctxp, pacc=pacc):
                        u = f"{st}_{h}"
                        rspp = pp.tile([1, 512], F32, tag="row", bufs=1,
                                       name=f"rsp_{u}")
                        nc.tensor.matmul(rspp[:], ones_col[:], pacc[:],
                                         start=True, stop=True)
                        rows = phase1.rows
                        nc.vector.reciprocal_approx_fast(rows[0:1, 3072:3584],
                                                         rspp[:])
                        rcpb = work.tile([128, 512], F32, tag="rcpb", bufs=2,
                                         name=f"rcpb_{u}")
                        nc.gpsimd.partition_broadcast(rcpb[:],
                                                      rows[0:1, 3072:3584])
                        cs = work.tile([128, 512], BF16, tag=f"cs{h}", bufs=2,
                                       name=f"cs_{u}")
                        nc.vector.tensor_mul(cs[:], ctxp[:], rcpb[:])
                        csb2[h] = cs
                    pend_tail.append(tail)

                    if h == HQL - 1:
                        flush_tail()
                        while zjobs:
                            zjobs.pop(0)()
                    for _ in range(per):
                        if jobs1:
                            jobs1.pop(0)()
                while jobs1:
                    jobs1.pop(0)()
                while zjobs:
                    zjobs.pop(0)()
                flush_tail()
                csb.clear()
                csb.update(csb2)
                csb2.clear()

            oseg_cur = [None]

            def phase3_dc(st, tt, dc, split_dma=False):
                """One [128,512] out-proj column block: 4 accumulating
                matmuls + a DVE copy; DMA on the last block of a row."""
                t0 = (st * TPS + tt) * 128
                if dc == 0:
                    oseg_cur[0] = work.tile([128, 2048], BF16, tag="oseg",
                                            bufs=3, name=f"oseg_{st}_{tt}")
                oseg = oseg_cur[0]
                op = pp.tile([128, 512], F32, tag="stp", bufs=3,
                             name=f"op_{st}_{tt}_{dc}")
                for h in range(HQL):
                    nc.tensor.matmul(
                        op[:],
                        csb[h][:, tt * 128:(tt + 1) * 128],
                        wo_sb[:, h * D + dc * 512:h * D + (dc + 1) * 512],
                        start=(h == 0), stop=(h == HQL - 1))
                if (tt * 4 + dc) % 2 == 1:
                    nc.vector.tensor_copy(oseg[:, dc * 512:(dc + 1) * 512],
                                          op[:])
                else:
                    nc.scalar.copy(oseg[:, dc * 512:(dc + 1) * 512], op[:])
                if split_dma and dc == 1:
                    nc.sync.dma_start(out=out[t0:t0 + 128, 0:1024],
                                      in_=oseg[:, 0:1024])
                elif dc == 3:
                    if split_dma:
                        nc.sync.dma_start(out=out[t0:t0 + 128, 1024:2048],
                                          in_=oseg[:, 1024:2048])
                    else:
                        nc.sync.dma_start(out=out[t0:t0 + 128, :], in_=oseg[:])

            def phase3_tt(st, tt, zipped=True):
                for dc in range(4):
                    phase3_dc(st, tt, dc)

            def phase3_jobs(st):
                return [(lambda st=st, tt=tt, dc=dc: phase3_dc(st, tt, dc))
                        for tt in range(TPS) for dc in range(4)]

            def phase3_final():
                """Last supertile's out-proj: open three h0..h2 partial
                accumulations first so the PE has work while head 3's
                softmax tail (rsp->recip->broadcast->cs mul) completes."""
                st = NST - 1
                oseg_cur[0] = work.tile([128, 2048], BF16, tag="oseg",
                                        bufs=3, name=f"oseg_{st}_0")
                ops = []
                for dc in range(3):
                    op = pp.tile([128, 512], F32, tag="stp", bufs=3,
                                 name=f"opf_{dc}")
                    for h in range(3):
                        nc.tensor.matmul(
                            op[:], csb[h][:, 0:128],
                            wo_sb[:, h * D + dc * 512:h * D + (dc + 1) * 512],
                            start=(h == 0), stop=False)
                    ops.append(op)
                oseg = oseg_cur[0]
                for dc in range(3):
                    nc.tensor.matmul(
                        ops[dc][:], csb[3][:, 0:128],
                        wo_sb[:, 3 * D + dc * 512:3 * D + (dc + 1) * 512],
                        start=False, stop=True)
                    if dc % 2 == 1:
                        nc.vector.tensor_copy(
                            oseg[:, dc * 512:(dc + 1) * 512], ops[dc][:])
                    else:
                        nc.scalar.copy(oseg[:, dc * 512:(dc + 1) * 512],
                                       ops[dc][:])
                phase3_dc(st, 0, 3)
                for tt in range(1, TPS):
                    for dc in range(4):
                        phase3_dc(st, tt, dc, split_dma=(tt == TPS - 1))

            phase1(0)
            # xst1 and wo aren't needed until phase2(0)'s interleaved jobs
            # (~45us) and phase3 (~60us); gating them on stq (the tail of
            # the gpsimd startup stream) keeps the bandwidth-saturated
            # startup window for the tensors phase1 actually blocks on.
            stqtail = stq[0:1, T - 16:T]
            load_xst(1, gate16=stqtail)
            nc.gpsimd.partition_broadcast(wo_sb[:, 0:16], stqtail)
            nc.sync.dma_start(
                out=wo_sb[:].rearrange("p (h d) -> p h d", h=HQL),
                in_=wo.rearrange("(h p) d -> p h d", p=128))
            phase2(0, jobs1=phase1_jobs(1))
            for st in range(1, NST):
                if st + 1 < NST:
                    load_xst(st + 1)
                phase2(st, zip3=st - 1,
                       jobs1=phase1_jobs(st + 1) if st + 1 < NST else None)
            phase3_final()

    if not nc.is_finalized():
        nc.finalize()
    return nc


def _prep_inputs(x, cos, sin, Wq, Wk, Wv, Wo, q_scale, k_scale):
    bf = ml_dtypes.bfloat16
    x = np.asarray(x, dtype=np.float32)
    cos = np.asarray(cos, dtype=np.float32)
    sin = np.asarray(sin, dtype=np.float32)
    qs = np.asarray(q_scale, dtype=np.float32)
    ks = np.asarray(k_scale, dtype=np.float32)
    d2 = HD // 2
    sgn = np.concatenate([-np.ones(d2, np.float32), np.ones(d2, np.float32)])
    qs_rot = np.concatenate([qs[d2:], qs[:d2]])
    ks_rot = np.concatenate([ks[d2:], ks[:d2]])
    # the kernel uses one shared table pair for q and k rope (saves 1MB of
    # bandwidth in the DMA-saturated startup window); q_scale/k_scale are
    # both ones(HD) in this problem
    assert np.allclose(qs, ks), "shared rope tables require q_scale==k_scale"
    ctq = np.ascontiguousarray((cos * qs[None, :]).T.astype(bf))
    stq = np.ascontiguousarray((sin * (sgn * qs_rot)[None, :]).T.astype(bf))
    ones_col = np.ones((128, 1), dtype=bf)
    ut01 = np.triu(np.ones((128, 128), np.float32)).astype(bf)

    in_maps = []
    for c in range(NCORES):
        b, g4 = c // 4, c % 4
        xTc = np.ascontiguousarray(x[b].T.astype(bf))
        WqT = np.ascontiguousarray(Wq[g4 * EQ:(g4 + 1) * EQ, :].T.astype(bf))
        WkT = Wk[g4 * EKV:(g4 + 1) * EKV, :].T
        WvT = Wv[g4 * EKV:(g4 + 1) * EKV, :].T
        Wkvc = np.ascontiguousarray(
            np.concatenate([WkT, WvT], axis=1).astype(bf))
        WoT = np.ascontiguousarray(Wo[:, g4 * EQ:(g4 + 1) * EQ].T.astype(bf))
        in_maps.append({
            "xT": xTc, "wq": WqT, "wkv": Wkvc, "wo": WoT,
            "ctq": ctq, "stq": stq,
            "ones_col": ones_col, "ut01": ut01,
        })
    return in_maps


def kernel(x, mask, cos, sin, Wq, Wk, Wv, Wo, q_scale, k_scale, _trace=False):
    global _compiled
    from concourse.bass_utils import run_bass_kernel_spmd
    if _compiled is None:
        _compiled = _build()
    nc = _compiled
    in_maps = _prep_inputs(x, cos, sin, np.asarray(Wq, np.float32),
                           np.asarray(Wk, np.float32), np.asarray(Wv, np.float32),
                           np.asarray(Wo, np.float32), q_scale, k_scale)
    res = run_bass_kernel_spmd(nc, in_maps, list(range(NCORES)), trace=_trace)
    parts = [np.asarray(res.results[i]["out"], dtype=np.float32)
             for i in range(NCORES)]
    outv = np.stack([parts[0] + parts[1] + parts[2] + parts[3],
                     parts[4] + parts[5] + parts[6] + parts[7]])
    kernel.last_result = res
    return outv.astype(np.float32)



# revision 48
# speedup vs baseline: 1.0085x; 1.0085x over previous
"""GroupedQueryAttention (B=2,T=2048,D=2048,HQ=16,HKV=8,HD=128) on 8 trn2 cores.

Sharding: 2-way data-parallel over batch x 4-way tensor-parallel over KV
groups.  Core c: batch c//4, KV-group c%4 (4 Q heads, 2 KV heads).  Each
core computes a partial [T, D] output (its heads' contribution through
out_proj) in bf16; the host sums the 4 TP partials per batch in f32.

Design (all matmul operands bf16, PSUM f32):
- Q/K projected directly in transposed [d, t] layout (weight chunks as
  lhsT, resident x^T supertile as rhs) -> no PE transposes and no
  per-head PSUM->SBUF copies; V projected in [t, e] (x^T chunks as lhsT).
- RMS-norm stats via ACT Square + ones-column matmul (partition reduce);
  1/rms = ACT Sqrt + DVE reciprocal_approx_fast; the inverse row is
  partition-broadcast by gpsimd and folded into the rope tail multiply.
  The stats matmul of head i issues while head i+1's projection chain
  streams, so the PE never waits on the ACT square.
- rope in [d, t] layout with q/k_scale and the rotate-half sign baked
  into transposed cos/sin tables (host-precomputed, bf16).
- attention in ST layout: S^T chunks [k,q] -> ACT exp -> P^T (bf16)
  feeds ctx^T; the softmax denominator is a bf16 DVE running sum of the
  P^T chunks + ONE ones-column matmul per (head, supertile) (instead of
  re-streaming every chunk through the PE), deferred into the next
  head's S phase so it never waits on the DVE add chain; division via
  DVE reciprocal_approx_fast + gpsimd partition_broadcast, applied once
  to ctx^T.  Causality at 128-chunk granularity + 0/1 triangle multiply
  on the diagonal chunk.  S runs 2 chunks ahead of exp (stp bufs=3).
- HAM warmup matmuls at t=0 + startup DMAs spread across 4 engine
  queues so the first projections start at 2.4 GHz as soon as their
  chunks land.
- software-pipelined emission: supertile st's attention interleaves the
  previous supertile's out-projection (per head) and the next
  supertile's projection jobs, keeping the PE dense while ACT paces the
  exp chain.  Startup DMAs are chunked and dual-queued.
"""

import numpy as np
import ml_dtypes

import concourse.bass as bass
import concourse.bacc as bacc
import concourse.mybir as mybir
from concourse.tile import TileContext

F32 = mybir.dt.float32
F32R = mybir.dt.float32r
BF16 = mybir.dt.bfloat16
AFT = mybir.ActivationFunctionType
ALU = mybir.AluOpType

B, T, D = 2, 2048, 2048
HQ, HKV, HD = 16, 8, 128
EPS = 1e-6
NCORES = 8
HQL, HKVL = 4, 2
EQ, EKV = HQL * HD, HKVL * HD   # 512, 256
ND = D // 128
NST = 4
TPS = 4
NTT = NST * TPS
ISQ = float(1.0 / np.sqrt(np.float32(HD)))

_compiled = None


def _build():
    nc = bacc.Bacc()
    xT = nc.declare_dram_parameter("xT", [D, T], BF16, isOutput=False)
    wq = nc.declare_dram_parameter("wq", [D, EQ], BF16, isOutput=False)
    wkv = nc.declare_dram_parameter("wkv", [D, EQ], BF16, isOutput=False)
    wo = nc.declare_dram_parameter("wo", [EQ, D], BF16, isOutput=False)
    ctq_d = nc.declare_dram_parameter("ctq", [HD, T], BF16, isOutput=False)
    stq_d = nc.declare_dram_parameter("stq", [HD, T], BF16, isOutput=False)
    ones_d = nc.declare_dram_parameter("ones_col", [128, 1], BF16, isOutput=False)
    ut01_d = nc.declare_dram_parameter("ut01", [128, 128], BF16, isOutput=False)
    out = nc.declare_dram_parameter("out", [T, D], BF16, isOutput=True)

    with TileContext(nc) as tc:
        with (
            nc.allow_low_precision(reason="bf16 matmuls, bf16 softmax tiles"),
            tc.tile_pool(name="res", bufs=1) as res,
            tc.tile_pool(name="work", bufs=2) as work,
            tc.tile_pool(name="pp", bufs=1, space="PSUM") as pp,
        ):
            wq_sb = res.tile([128, ND * EQ], BF16, name="wq_sb")
            wkv_sb = res.tile([128, ND * EQ], BF16, name="wkv_sb")
            wo_sb = res.tile([128, HQL * D], BF16, name="wo_sb")
            ktr_sb = res.tile([128, HKVL * T], BF16, name="ktr_sb")
            vv_sb = res.tile([128, NTT * EKV], BF16, name="vv_sb")
            ctq = res.tile([128, T], BF16, name="ctq")
            stq = res.tile([128, T], BF16, name="stq")
            ones_col = res.tile([128, 1], BF16, name="ones_col")
            ut01 = res.tile([128, 128], BF16, name="ut01")
            zero128 = res.tile([128, 1], F32, name="zero128")
            ones_f32r = res.tile([128, 1], F32, name="ones_f32r")
            nc.vector.memset(ones_f32r[:], 1.0)
            eps1 = res.tile([1, 1], F32, name="eps1")
            nc.vector.memset(zero128[:], 0.0)
            nc.vector.memset(eps1[:], EPS)

            # HAM warmup: keep the PE busy while the startup DMAs land so
            # the real matmuls start at 2.4 GHz instead of 1.2.  Streams
            # uninitialized wo_sb garbage into a never-read PSUM tile;
            # wo's DMA is last in the startup order, so the WAR edge on
            # these reads costs nothing.
            wrm = pp.tile([128, 512], F32, tag="proj", bufs=2, name="warmpp")
            NWARM = 7
            for i in range(NWARM):
                nc.tensor.matmul(wrm[:], wo_sb[:, 0:128], wo_sb[:, 0:512],
                                 start=(i == 0), stop=(i == NWARM - 1))

            # startup DMAs: interleave x^T(st=0) chunks with Wq chunks so
            # the first projection matmuls can begin ASAP; everything else
            # lands while the first projections run.
            wqr = wq.rearrange("(j p) e -> p j e", p=128)
            wkvr = wkv.rearrange("(j p) e -> p j e", p=128)
            x0 = work.tile([128, ND * 512], BF16, tag="xst", bufs=2,
                           name="xst_0")
            x0r = x0[:].rearrange("p (j t) -> p j t", j=ND)
            xr0 = xT.rearrange("(j p) t -> p j t", p=128)[:, :, 0:512]
            wq_v = wq_sb[:].rearrange("p (j e) -> p j e", j=ND)
            wkv_v = wkv_sb[:].rearrange("p (j e) -> p j e", j=ND)
            # startup DMAs across the 3 DMA-capable queues: x on sync
            # (kept clear for the xst streams), wq on scalar, everything
            # else on gpsimd in consumption order.  ut01 and wo go last —
            # they are needed late and their descriptor-gen is expensive.
            for j0, jw in ((0, 2), (2, 2), (4, 4), (8, 4), (12, 4)):
                nc.sync.dma_start(out=x0r[:, j0:j0 + jw],
                                  in_=xr0[:, j0:j0 + jw])
                nc.scalar.dma_start(out=wq_v[:, j0:j0 + jw],
                                    in_=wqr[:, j0:j0 + jw])
            nc.gpsimd.dma_start(out=ones_col[:], in_=ones_d[:])
            # Gate the rest of the gpsimd DMA stream on x0 being nearly
            # landed: HBM bandwidth (~358 GB/s/core) is shared across
            # queues, and x0/wq are the tensors the first projections
            # block on.  The gate is a WAW data dep: a tiny broadcast of
            # x0's last chunk into each DMA's destination, so the
            # scheduler cannot hoist the DMA ahead of it.
            x0tail = x0[0:1, 15 * 512:15 * 512 + 16]

            def dgate(dst16):
                nc.gpsimd.partition_broadcast(dst16, x0tail)

            # rope tables first: the q-rope DVE muls free the proj PSUM
            # buffers that the k projections block on
            dgate(ctq[:, 0:16])
            nc.gpsimd.dma_start(out=ctq[:], in_=ctq_d[:])
            dgate(stq[:, 0:16])
            nc.gpsimd.dma_start(out=stq[:], in_=stq_d[:])
            for j0 in range(0, ND, 4):
                dgate(wkv_sb[:, j0 * 512:j0 * 512 + 16])
                nc.gpsimd.dma_start(out=wkv_v[:, j0:j0 + 4],
                                    in_=wkvr[:, j0:j0 + 4])
            nc.gpsimd.dma_start(out=ut01[:], in_=ut01_d[:])

            xst = [None] * NST
            xst[0] = x0
            qtr = [None] * HQL
            csb = {}
            csb2 = {}

            def load_xst(st, gate16=None):
                t = work.tile([128, ND * 512], BF16, tag="xst", bufs=2,
                              name=f"xst_{st}")
                tr = t[:].rearrange("p (j t) -> p j t", j=ND)
                xr = xT.rearrange("(j p) t -> p j t",
                                  p=128)[:, :, st * 512:(st + 1) * 512]
                for j0 in range(0, ND, 4):
                    if gate16 is not None:
                        nc.gpsimd.partition_broadcast(
                            t[:, j0 * 512:j0 * 512 + 16], gate16)
                    nc.sync.dma_start(out=tr[:, j0:j0 + 4], in_=xr[:, j0:j0 + 4])
                xst[st] = t

            def proj_head(st, wsb, eoff, name):
                """16-chunk projection matmul into a [128,512] PSUM tile."""
                prj = pp.tile([128, 512], F32, tag="proj", bufs=2, name=name)
                for j in range(ND):
                    nc.tensor.matmul(
                        prj[:],
                        wsb[:, j * EQ + eoff:j * EQ + eoff + 128],
                        xst[st][:, j * 512:(j + 1) * 512],
                        start=(j == 0), stop=(j == ND - 1))
                return prj

            def rope_early(st, hh, prj, ct, st_t, acc):
                """ACT square + inv-independent rope part."""
                u = f"{st}_{hh}"
                c0 = st * 512
                sq = work.tile([128, 512], BF16, tag="sq", bufs=3, name=f"sq_{u}")
                nc.scalar.activation(sq[:], prj[:], AFT.Square,
                                     bias=zero128[:, 0:1])
                tmp = work.tile([128, 512], BF16, tag="tmp", bufs=2,
                                name=f"tmp_{u}")
                nc.vector.tensor_mul(tmp[0:64, :], prj[64:128, :],
                                     st_t[0:64, c0:c0 + 512])
                nc.vector.tensor_mul(tmp[64:128, :], prj[0:64, :],
                                     st_t[64:128, c0:c0 + 512])
                nc.vector.tensor_mul(acc[:], prj[:], ct[:, c0:c0 + 512])
                nc.vector.tensor_add(acc[:], acc[:], tmp[:])
                return sq

            def stats_late(st, hh, sq, cat_row, cslot):
                u = f"{st}_{hh}"
                stat = pp.tile([1, 512], F32, tag="row", bufs=1, name=f"st_{u}")
                nc.tensor.matmul(stat[:], ones_col[:], sq[:], start=True,
                                 stop=True)
                nc.scalar.activation(cat_row[0:1, cslot:cslot + 512], stat[:],
                                     AFT.Sqrt, bias=eps1[:, 0:1], scale=1.0 / HD)

            def inv_chain(cat, c0, width, rows, invb, io):
                """invb[:, io:io+width] = broadcast(1/cat[0, c0:c0+width])."""
                nc.vector.reciprocal_approx_fast(rows[0:1, c0:c0 + width],
                                                 cat[0:1, c0:c0 + width])
                nc.gpsimd.partition_broadcast(invb[:, io:io + width],
                                              rows[0:1, c0:c0 + width])

            def phase1_jobs(st):
                """Return a list of emission closures (jobs) for supertile
                st's projections+norm+rope; run in order, possibly
                interleaved into phase2's head loop."""
                cat = work.tile([1, 3584], F32, tag="cat", bufs=1,
                                name=f"cat_{st}")
                rows = work.tile([1, 3584], F32, tag="rows", bufs=1,
                                 name=f"rows_{st}")
                invb = work.tile([128, 3072], F32, tag="invb", bufs=1,
                                 name=f"invb_{st}")
                pend = []

                def flush_pend():
                    if not pend:
                        return
                    kind, idx, sq2, acc2 = pend.pop(0)
                    cslot = idx * 512 if kind == "q" else 2048 + idx * 512
                    stats_late(st, f"{kind}{idx}", sq2, cat, cslot)
                    inv_chain(cat, cslot, 512, rows, invb, cslot)
                    if kind == "q":
                        q = work.tile([128, 512], BF16, tag=f"qtr{idx}", bufs=2,
                                      name=f"qtr_{st}_{idx}")
                        nc.vector.tensor_mul(q[:], acc2[:],
                                             invb[:, cslot:cslot + 512])
                        qtr[idx] = q
                    else:
                        nc.vector.tensor_mul(
                            ktr_sb[:, idx * T + st * 512:idx * T + (st + 1) * 512],
                            acc2[:], invb[:, cslot:cslot + 512])

                def qk_job(kind, idx):
                    def run():
                        if kind == "q":
                            prj = proj_head(st, wq_sb, idx * 128,
                                            f"qp_{st}_{idx}")
                            acc = work.tile([128, 512], BF16, tag=f"qacc{idx}",
                                            bufs=2, name=f"qacc_{st}_{idx}")
                            sq = rope_early(st, f"q{idx}", prj, ctq, stq, acc)
                        else:
                            prj = proj_head(st, wkv_sb, idx * 128,
                                            f"kp_{st}_{idx}")
                            acc = work.tile([128, 512], BF16, tag=f"kacc{idx}",
                                            bufs=2, name=f"kacc_{st}_{idx}")
                            sq = rope_early(st, f"k{idx}", prj, ctq, stq, acc)
                        flush_pend()
                        pend.append((kind, idx, sq, acc))
                    return run

                def v_job(tq):
                    def run():
                        c = st * TPS + tq
                        vp = pp.tile([128, 512], F32, tag="proj", bufs=2,
                                     name=f"vp_{st}_{tq}")
                        for j in range(ND):
                            nc.tensor.matmul(
                                vp[:, 0:EKV],
                                xst[st][:, j * 512 + tq * 128:
                                        j * 512 + (tq + 1) * 128],
                                wkv_sb[:, j * EQ + EKV:(j + 1) * EQ],
                                start=(j == 0), stop=(j == ND - 1))
                        nc.scalar.copy(vv_sb[:, c * EKV:(c + 1) * EKV],
                                       vp[:, 0:EKV])
                        if tq == 0:
                            flush_pend()
                        elif tq == 1:
                            while pend:
                                flush_pend()
                    return run

                def qq012_job():
                    """q0+q1+q2 chunk-major: the startup is DMA-paced, so
                    interleave three heads' j-chunks to consume each
                    arriving x/wq chunk with 3x the PE work."""
                    prjs = [pp.tile([128, 512], F32, tag="proj", bufs=2,
                                    name=f"qqp_{i}") for i in range(2)]
                    prjs.append(pp.tile([128, 512], F32, tag="stp", bufs=3,
                                        name="qqp_2"))
                    for j in range(ND):
                        for i in range(3):
                            nc.tensor.matmul(
                                prjs[i][:],
                                wq_sb[:, j * EQ + i * 128:j * EQ + i * 128 + 128],
                                xst[0][:, j * 512:(j + 1) * 512],
                                start=(j == 0), stop=(j == ND - 1))
                    for i in range(3):
                        acc = work.tile([128, 512], BF16, tag=f"qacc{i}",
                                        bufs=2, name=f"qacc_0_{i}")
                        sq = rope_early(0, f"q{i}", prjs[i], ctq, stq, acc)
                        flush_pend()
                        pend.append(("q", i, sq, acc))

                phase1.rows = rows
                if st == 0:
                    jobs = [qq012_job]
                    jobs += [qk_job("q", h) for h in range(3, HQL)]
                else:
                    jobs = [qk_job("q", h) for h in range(HQL)]
                jobs += [qk_job("k", g) for g in range(HKVL)]
                jobs += [v_job(tq) for tq in range(TPS)]
                return jobs

            def phase1(st):
                for j in phase1_jobs(st):
                    j()

            def phase2(st, zip3=None, jobs1=None):
                nch = TPS * (st + 1)
                jobs1 = list(jobs1 or [])
                per = (len(jobs1) + HQL - 1) // HQL if jobs1 else 0
                qcur = list(qtr)
                pend_tail = []
                # previous supertile's out-proj, interleaved one [128,512]
                # block at a time through the chunk loop: those ~900ns PE
                # fillers cover the ACT exp deficit (exp ~571ns/chunk vs
                # ~426ns of S+ctx PE work).
                zjobs = phase3_jobs(zip3) if zip3 is not None else []
                znum = len(zjobs)
                zticks = HQL * nch
                zctr = [0]

                def ztick():
                    zctr[0] += 1
                    # proportional pacing: job k fires at tick k*zticks/znum
                    while zjobs and zctr[0] * znum >= \
                            (znum - len(zjobs) + 1) * zticks:
                        zjobs.pop(0)()

                def flush_tail():
                    if pend_tail:
                        pend_tail.pop()()

                for h in range(HQL):
                    g = h // 2
                    ctxp = pp.tile([128, 512], F32, tag="ctx", bufs=2,
                                   name=f"ctx_{st}_{h}")
                    # bf16 running sum of P^T chunks on the DVE; the final
                    # [1,512] denominator is ONE ones-column matmul instead
                    # of re-streaming every chunk through the PE.  bf16 is
                    # enough: the f32 partition-reduce averages the
                    # independent per-entry roundings (all-positive sums).
                    pacc = work.tile([128, 512], BF16, tag="pacc", bufs=2,
                                     name=f"pacc_{st}_{h}")
                    pts = [None] * nch

                    def s_exp(ci):
                        dj = ci - TPS * st
                        qlo = 128 * dj if dj > 0 else 0
                        w = 512 - qlo
                        stp = pp.tile([128, 512], F32, tag="stp", bufs=3,
                                      name=f"stp_{st}_{h}_{ci}")
                        nc.tensor.matmul(
                            stp[:, :w],
                            ktr_sb[:, g * T + ci * 128:g * T + (ci + 1) * 128],
                            qcur[h][:, qlo:512],
                            start=True, stop=True)
                        pt = work.tile([128, 512], BF16, tag="pt", bufs=4,
                                       name=f"pt_{st}_{h}_{ci}")
                        nc.scalar.activation(pt[:, :w], stp[:, :w], AFT.Exp,
                                             bias=zero128[:, 0:1], scale=ISQ)
                        if dj >= 0:
                            nc.vector.tensor_mul(pt[:, :128], pt[:, :128],
                                                 ut01[:])
                        if ci == 0:
                            nc.vector.tensor_copy(pacc[:], pt[:])
                        else:
                            nc.vector.tensor_add(pacc[:, qlo:512],
                                                 pacc[:, qlo:512], pt[:, :w])
                        pts[ci] = pt

                    def ctx_rsp(ci):
                        dj = ci - TPS * st
                        qlo = 128 * dj if dj > 0 else 0
                        w = 512 - qlo
                        first, last = ci == 0, ci == nch - 1
                        nc.tensor.matmul(
                            ctxp[:, qlo:512],
                            vv_sb[:, ci * EKV + g * 128:ci * EKV + (g + 1) * 128],
                            pts[ci][:, :w],
                            start=first, stop=last)
                        pts[ci] = None

                    la = min(2, nch - 1)
                    for ci in range(la):
                        s_exp(ci)
                    # previous head's denominator + normalize, deferred here
                    # so its rsp matmul never waits on the DVE add chain
                    flush_tail()
                    for ci in range(la, nch):
                        s_exp(ci)
                        ctx_rsp(ci - la)
                        ztick()
                    for ci in range(nch - la, nch):
                        ctx_rsp(ci)
                        ztick()

                    def tail(h=h, ctxp=ctxp, pacc=pacc):
                        u = f"{st}_{h}"
                        rspp = pp.tile([1, 512], F32, tag="row", bufs=1,
                                       name=f"rsp_{u}")
                        nc.tensor.matmul(rspp[:], ones_col[:], pacc[:],
                                         start=True, stop=True)
                        rows = phase1.rows
                        nc.vector.reciprocal_approx_fast(rows[0:1, 3072:3584],
                                                         rspp[:])
                        rcpb = work.tile([128, 512], F32, tag="rcpb", bufs=2,
                                         name=f"rcpb_{u}")
                        nc.gpsimd.partition_broadcast(rcpb[:],
                                                      rows[0:1, 3072:3584])
                        cs = work.tile([128, 512], BF16, tag=f"cs{h}", bufs=2,
                                       name=f"cs_{u}")
                        nc.vector.tensor_mul(cs[:], ctxp[:], rcpb[:])
                        csb2[h] = cs
                    pend_tail.append(tail)

                    if h == HQL - 1:
                        flush_tail()
                        while zjobs:
                            zjobs.pop(0)()
                    for _ in range(per):
                        if jobs1:
                            jobs1.pop(0)()
                while jobs1:
                    jobs1.pop(0)()
                while zjobs:
                    zjobs.pop(0)()
                flush_tail()
                csb.clear()
                csb.update(csb2)
                csb2.clear()

            oseg_cur = [None]

            def phase3_dc(st, tt, dc, split_dma=False):
                """One [128,512] out-proj column block: 4 accumulating
                matmuls + a DVE copy; DMA on the last block of a row."""
                t0 = (st * TPS + tt) * 128
                if dc == 0:
                    oseg_cur[0] = work.tile([128, 2048], BF16, tag="oseg",
                                            bufs=3, name=f"oseg_{st}_{tt}")
                oseg = oseg_cur[0]
                op = pp.tile([128, 512], F32, tag="stp", bufs=3,
                             name=f"op_{st}_{tt}_{dc}")
                for h in range(HQL):
                    nc.tensor.matmul(
                        op[:],
                        csb[h][:, tt * 128:(tt + 1) * 128],
                        wo_sb[:, h * D + dc * 512:h * D + (dc + 1) * 512],
                        start=(h == 0), stop=(h == HQL - 1))
                if (tt * 4 + dc) % 2 == 1:
                    nc.vector.tensor_copy(oseg[:, dc * 512:(dc + 1) * 512],
                                          op[:])
                else:
                    nc.scalar.copy(oseg[:, dc * 512:(dc + 1) * 512], op[:])
                if split_dma and dc == 1:
                    nc.sync.dma_start(out=out[t0:t0 + 128, 0:1024],
                                      in_=oseg[:, 0:1024])
                elif dc == 3:
                    if split_dma:
                        nc.sync.dma_start(out=out[t0:t0 + 128, 1024:2048],
                                          in_=oseg[:, 1024:2048])
                    else:
                        nc.sync.dma_start(out=out[t0:t0 + 128, :], in_=oseg[:])

            def phase3_tt(st, tt, zipped=True):
                for dc in range(4):
                    phase3_dc(st, tt, dc)

            def phase3_jobs(st):
                return [(lambda st=st, tt=tt, dc=dc: phase3_dc(st, tt, dc))
                        for tt in range(TPS) for dc in range(4)]

            def phase3_final():
                """Last supertile's out-proj: open three h0..h2 partial
                accumulations first so the PE has work while head 3's
                softmax tail (rsp->recip->broadcast->cs mul) completes."""
                st = NST - 1
                oseg_cur[0] = work.tile([128, 2048], BF16, tag="oseg",
                                        bufs=3, name=f"oseg_{st}_0")
                ops = []
                for dc in range(3):
                    op = pp.tile([128, 512], F32, tag="stp", bufs=3,
                                 name=f"opf_{dc}")
                    for h in range(3):
                        nc.tensor.matmul(
                            op[:], csb[h][:, 0:128],
                            wo_sb[:, h * D + dc * 512:h * D + (dc + 1) * 512],
                            start=(h == 0), stop=False)
                    ops.append(op)
                oseg = oseg_cur[0]
                for dc in range(3):
                    nc.tensor.matmul(
                        ops[dc][:], csb[3][:, 0:128],
                        wo_sb[:, 3 * D + dc * 512:3 * D + (dc + 1) * 512],
                        start=False, stop=True)
                    if dc % 2 == 1:
                        nc.vector.tensor_copy(
                            oseg[:, dc * 512:(dc + 1) * 512], ops[dc][:])
                    else:
                        nc.scalar.copy(oseg[:, dc * 512:(dc + 1) * 512],
                                       ops[dc][:])
                phase3_dc(st, 0, 3)
                for tt in range(1, TPS):
                    for dc in range(4):
                        phase3_dc(st, tt, dc, split_dma=(tt == TPS - 1))

            phase1(0)
            # xst1 and wo aren't needed until phase2(0)'s interleaved jobs
            # (~45us) and phase3 (~60us); gating them on stq (the tail of
            # the gpsimd startup stream) keeps the bandwidth-saturated
            # startup window for the tensors phase1 actually blocks on.
            stqtail = stq[0:1, T - 16:T]
            load_xst(1, gate16=stqtail)
            nc.gpsimd.partition_broadcast(wo_sb[:, 0:16], stqtail)
            nc.sync.dma_start(
                out=wo_sb[:].rearrange("p (h d) -> p h d", h=HQL),
                in_=wo.rearrange("(h p) d -> p h d", p=128))
            phase2(0, jobs1=phase1_jobs(1))
            for st in range(1, NST):
                if st + 1 < NST:
                    load_xst(st + 1)
                phase2(st, zip3=st - 1,
                       jobs1=phase1_jobs(st + 1) if st + 1 < NST else None)
            phase3_final()

    if not nc.is_finalized():
        nc.finalize()
    return nc


def _prep_inputs(x, cos, sin, Wq, Wk, Wv, Wo, q_scale, k_scale):
    bf = ml_dtypes.bfloat16
    x = np.asarray(x, dtype=np.float32)
    cos = np.asarray(cos, dtype=np.float32)
    sin = np.asarray(sin, dtype=np.float32)
    qs = np.asarray(q_scale, dtype=np.float32)
    ks = np.asarray(k_scale, dtype=np.float32)
    d2 = HD // 2
    sgn = np.concatenate([-np.ones(d2, np.float32), np.ones(d2, np.float32)])
    qs_rot = np.concatenate([qs[d2:], qs[:d2]])
    ks_rot = np.concatenate([ks[d2:], ks[:d2]])
    # the kernel uses one shared table pair for q and k rope (saves 1MB of
    # bandwidth in the DMA-saturated startup window); q_scale/k_scale are
    # both ones(HD) in this problem
    assert np.allclose(qs, ks), "shared rope tables require q_scale==k_scale"
    ctq = np.ascontiguousarray((cos * qs[None, :]).T.astype(bf))
    stq = np.ascontiguousarray((sin * (sgn * qs_rot)[None, :]).T.astype(bf))
    ones_col = np.ones((128, 1), dtype=bf)
    ut01 = np.triu(np.ones((128, 128), np.float32)).astype(bf)

    in_maps = []
    for c in range(NCORES):
        b, g4 = c // 4, c % 4
        xTc = np.ascontiguousarray(x[b].T.astype(bf))
        WqT = np.ascontiguousarray(Wq[g4 * EQ:(g4 + 1) * EQ, :].T.astype(bf))
        WkT = Wk[g4 * EKV:(g4 + 1) * EKV, :].T
        WvT = Wv[g4 * EKV:(g4 + 1) * EKV, :].T
        Wkvc = np.ascontiguousarray(
            np.concatenate([WkT, WvT], axis=1).astype(bf))
        WoT = np.ascontiguousarray(Wo[:, g4 * EQ:(g4 + 1) * EQ].T.astype(bf))
        in_maps.append({
            "xT": xTc, "wq": WqT, "wkv": Wkvc, "wo": WoT,
            "ctq": ctq, "stq": stq,
            "ones_col": ones_col, "ut01": ut01,
        })
    return in_maps


def kernel(x, mask, cos, sin, Wq, Wk, Wv, Wo, q_scale, k_scale, _trace=False):
    global _compiled
    from concourse.bass_utils import run_bass_kernel_spmd
    if _compiled is None:
        _compiled = _build()
    nc = _compiled
    in_maps = _prep_inputs(x, cos, sin, np.asarray(Wq, np.float32),
                           np.asarray(Wk, np.float32), np.asarray(Wv, np.float32),
                           np.asarray(Wo, np.float32), q_scale, k_scale)
    res = run_bass_kernel_spmd(nc, in_maps, list(range(NCORES)), trace=_trace)
    parts = [np.asarray(res.results[i]["out"], dtype=np.float32)
             for i in range(NCORES)]
    outv = np.stack([parts[0] + parts[1] + parts[2] + parts[3],
                     parts[4] + parts[5] + parts[6] + parts[7]])
    kernel.last_result = res
    return outv.astype(np.float32)

